# revision 28
# baseline (speedup 1.0000x reference)
"""Contrastive-learning loss kernel for 8 TRN2 NeuronCores (fp8, round 3).

loss = logsumexp(sim_neg / T) - mean(sim_pos) / T

Two reductions:
  denom = sum_ij exp(s_i . r_j / (T ||s_i|| ||r_j||))   (full N x N matmul)
  sum(sim_pos) = (sum_i s_i/||s_i||) . (sum_j b_j/||b_j||)  (rank-1 identity)

Sharding: 2 x 4 grid over the sim_neg matrix. Core c = a*4+b gets the
s-block rows [a*2048, (a+1)*2048) and r-block rows [b*1024, (b+1)*1024),
plus the c-th 512-row shard of x_source / x_bc_target for the numerator
partials (host-side row rotation puts each core's shard first so one
program serves all cores). Host combines partial exp-sums and weighted
row-sums in f64.

Round-3 notes, driven by round-2 traces (111 us baseline): the PE issue
rate for fp8 DoubleRow [K=256, 128x512] is 215 ns = the 157 TF/s wall,
so the 256 main matmuls are an irreducible ~55 us and the kernel's job
is to keep the PE streaming them back-to-back (every stall also drops
the PE to its 1.2 GHz p-state for ~3 us). Hence:

  - NO gram matmuls: s row norms come from fused square+reduce over a
    row-major fp8 copy of the s-block, spread across DVE and ACT which
    both have idle capacity (a gram pass is weight-load-bound and eats
    ~22 us of PE no matter how it is scheduled).
  - all 1/sqrt chains run as Exp(-0.5*Ln(x)) on ACT: no slow DVE
    reciprocals, and {Ln, Exp, Square, Copy} live in ONE activation
    table (round 2 paid 4 ACT_TABLE_LOAD swaps).
  - main loop works [128, 1024] tiles: each s-chunk's two 512-col psum
    halves are one 2-bank psum tile, consumed by ONE DVE multiply
    (f32 psum -> bf16) and ONE exp+accum whose per-partition scale is
    the chunk's s-norm and whose accumulator is the denominator col.
  - the first DEFER chunks evacuate psum as raw bf16 sims so the PE
    ring never waits for the r-norm (rinvb) or s-norm chains.
  - input DMA is split across both hardware DGE queues (each FIFOs at
    ~210 GB/s) with the first s pair-blocks at both heads, then the r
    slices in square order, then everything else in consumption order.

fp8 error analysis: cosine sims are ~N(0, 1/2048); e4m3 quantization
perturbs each sim by ~6% relative, which shifts log(denom) by ~1e-5 --
five orders of magnitude inside the 2e-2 tolerance (measured 6.9e-7
end to end with this arithmetic).
"""

import json

import numpy as np

import concourse.bass as bass
import concourse.mybir as mybir
import concourse.tile as tile
from concourse.bass_utils import run_bass_kernel_spmd

P = 128
N = 4096
D = 2048
TEMP = 0.5
A_SPLIT = 2  # s-row blocks
B_SPLIT = 4  # r-row blocks
SB = N // A_SPLIT  # 2048 source rows per core
RB = N // B_SPLIT  # 1024 raw-target rows per core
NSH = N // 8  # 512 numerator-shard rows per core
KU = D // (2 * P)  # 8 DoubleRow contraction pairs
SCN = SB // P  # 16 stationary s chunks of 128
NT = NSH // P  # 4 numerator-shard tiles
SP2 = 8  # pair-chunk s DMA blocks
DEFER = 4  # chunks whose psum is evacuated raw (norm chains not ready)

F32 = mybir.dt.float32
F8 = mybir.dt.float8e4
BF16 = mybir.dt.bfloat16
AF = mybir.ActivationFunctionType
DR = mybir.MatmulPerfMode.DoubleRow
ALU = mybir.AluOpType


def _spill_sync_waits(raw: bytes) -> bytes:
    """The walrus here has no sync-wait slots on Matmult (fused weight-load
    S3_LW struct) and chokes on multi-wait instructions generally. Move every
    Matmult wait -- and all but the first wait of any other instruction --
    onto single-wait NoOps inserted just before it on the same engine
    queue."""
    d = json.loads(raw)
    ctr = 0
    for fn in d["functions"]:
        for blk in fn["blocks"]:
            out = []
            for inst in blk["instructions"]:
                si = inst.get("sync_info")
                waits = si.get("on_wait") if si else None
                limit = 0 if inst.get("opcode") == "Matmult" else 1
                if waits and len(waits) > limit:
                    for w in waits[limit:]:
                        ctr += 1
                        out.append(
                            {
                                "debug": inst.get("debug"),
                                "engine": inst["engine"],
                                "ins": [],
                                "name": f"I-waitfix-{ctr}",
                                "opcode": "NoOp",
                                "outs": [],
                                "sync_info": {"on_update": [], "on_wait": [w]},
                            }
                        )
                    si["on_wait"] = waits[:limit]
                out.append(inst)
            blk["instructions"] = out
    return json.dumps(d).encode()


class PatchedBass(bass.Bass):
    def to_json_bytes(self) -> bytes:
        return _spill_sync_waits(super().to_json_bytes())


class TC(tile.TileContext):
    """TileContext whose kernel-tail drain carries its sem waits on
    single-wait NOPs -- this walrus rejects multi-wait Drain instructions."""

    def _drain_and_barrier(self, tick_clock, wait_clock):
        from concourse.vector_clock import ScopedClock, VectorClock

        g = tick_clock.global_clock
        nprocs = len(g)
        for p in range(nprocs):
            t = g[p]
            if t <= 0:
                continue
            vec = [0] * nprocs
            vec[p] = t
            nop = self.nc.sync.nop(nofuse=True)
            wait_clock.add_sem_waits(nop.ins, ScopedClock({None: VectorClock(vec)}))
        self.nc.sync.drain()
        self.nc.all_engine_barrier()
        assert self.sems is not None
        popped = self.nc._tile_sem_poison_stack.pop()
        assert popped is self._sem_poison
        self.nc.clear_and_free_semaphores(list(self.sems.allocated().values()))
        self.nc.all_engine_barrier()


def build():
    nc = PatchedBass()
    # sT8d: pair-chunk-blocked transposed s-block, k = (2u+t)*128 + p
    sT8d = nc.dram_tensor("sT8", [SP2, P, KU, 2, 2 * P], F8, kind="ExternalInput")
    # rT8d: per-u slices of the transposed r-block
    rT8d = nc.dram_tensor("rT8", [KU, P, 2, RB], F8, kind="ExternalInput")
    # sm8d: row-major s-block in partition-major groups of NT tiles
    # (rotated: tiles 0..3 are the numerator shard)
    sm8d = nc.dram_tensor("sm8", [SCN // NT, P, NT, D], F8, kind="ExternalInput")
    # sh8d/bh8d: row-major numerator shards, partition-major (sh8 is a
    # copy of sm8 tiles 0..3 so the numerator's reads never depend on the
    # big sm8 tile's last DMA)
    sh8d = nc.dram_tensor("sh8", [P, NT, D], F8, kind="ExternalInput")
    bh8d = nc.dram_tensor("bh8", [P, NT, D], F8, kind="ExternalInput")
    dacc_d = nc.dram_tensor("dacc", [P, SCN + 1], F32, kind="ExternalOutput")
    ssum_d = nc.dram_tensor("ssum", [1, D], F32, kind="ExternalOutput")
    bsum_d = nc.dram_tensor("bsum", [1, D], F32, kind="ExternalOutput")

    with TC(nc) as tc:
        with (
            tc.tile_pool(name="big", bufs=1) as big,
            tc.tile_pool(name="work", bufs=2) as work,
            tc.tile_pool(name="gpool", bufs=3, space="PSUM") as gpool,
            tc.tile_pool(name="rpool", bufs=1, space="PSUM") as rpool,
        ):
            sT8 = big.tile([P, SP2, KU, 2, 2 * P], F8, name="sT8")
            rT8 = big.tile([P, KU, 2, RB], F8, name="rT8")
            sm8 = big.tile([P, SCN, D], F8, name="sm8")
            sh8 = big.tile([P, NT, D], F8, name="sh8")
            bh8 = big.tile([P, NT, D], F8, name="bh8")
            dacc = big.tile([P, SCN + 1], F32, name="dacc")
            ones8 = big.tile([P, 2, P], F8, name="ones8")
            nc.vector.memset(ones8, 1.0)

            # ---- DMAs. sync queue: first s pair + r evens + early sm8
            # groups + late s pairs; ACT queue: second s pair + r odds +
            # mid s pairs + bh8 + remaining sm8.
            # All input triggers ride the Sync engine: a hardware-DGE
            # trigger past the queue depth WAITS for a prior transfer's
            # completion, and on the ACT queue that head-of-line blocked
            # every activation behind it for ~15 us. Sync has nothing
            # better to do.
            # The scalar queue gets exactly six triggers -- within the DGE
            # depth, so none of them parks a completion-wait in front of
            # ACT's compute stream -- carrying the tensors consumed last.
            nc.scalar.dma_start(out=sm8[:, 0:NT], in_=sm8d[0])
            nc.scalar.dma_start(out=sm8[:, NT : 2 * NT], in_=sm8d[1])
            nc.scalar.dma_start(out=sT8[:, 5], in_=sT8d[5])
            nc.scalar.dma_start(out=sT8[:, 6], in_=sT8d[6])
            nc.scalar.dma_start(out=sT8[:, 7], in_=sT8d[7])
            nc.scalar.dma_start(out=sm8[:, 3 * NT : SCN], in_=sm8d[3])
            nc.sync.dma_start(out=sT8[:, 0], in_=sT8d[0])
            for u in range(KU):
                nc.sync.dma_start(out=rT8[:, u], in_=rT8d[u])
            nc.sync.dma_start(out=sT8[:, 1], in_=sT8d[1])
            nc.sync.dma_start(out=sT8[:, 2], in_=sT8d[2])
            nc.sync.dma_start(out=sh8, in_=sh8d[:, :, :])
            nc.sync.dma_start(out=sT8[:, 3], in_=sT8d[3])
            nc.sync.dma_start(out=sT8[:, 4], in_=sT8d[4])
            nc.sync.dma_start(out=sm8[:, 2 * NT : 3 * NT], in_=sm8d[2])
            nc.sync.dma_start(out=bh8, in_=bh8d[:, :, :])

            # ---- r-column ssq: square the transposed r slices (DVE takes
            # evens, ACT odds, matching arrival), then DoubleRow
            # ones-matmuls reduce partitions; the [128, 1024] psum output
            # holds the column sums on every partition.
            rsqp = rpool.tile([P, RB], F32, name="rsqp")
            # Warm the PE's p-state during the input-DMA window: ~4 us of
            # dummy ones-matmuls (the real r-ones group later resets the
            # bank with start=True). Cold matmuls otherwise run at 1.2 GHz
            # for the first ~3 us of the real stream.
            wsrc = big.tile([P, 2, 512], F8, name="wsrc")
            nc.vector.memset(wsrc, 0.0)
            for _ in range(18):
                nc.tensor.matmul(
                    rsqp[:, 0:512],
                    lhsT=ones8,
                    rhs=wsrc,
                    start=True,
                    stop=True,
                    perf_mode=DR,
                )
            sqrs = []
            for u in range(KU):
                sqr = work.tile([P, 2, RB], F8, tag="sqr", bufs=KU, name="sqr")
                with nc.allow_low_precision(reason="fp8 squares"):
                    if u % 2 == 0:
                        nc.vector.tensor_mul(sqr, rT8[:, u], rT8[:, u])
                    else:
                        nc.scalar.activation(out=sqr, in_=rT8[:, u], func=AF.Square)
                sqrs.append(sqr)

            def r_ones(h):
                # emitted mid-loop so these matmuls never head-of-line
                # block the sim stream behind the squares
                for u in range(KU):
                    nc.tensor.matmul(
                        rsqp[:, h * 512 : (h + 1) * 512],
                        lhsT=ones8,
                        rhs=sqrs[u][:, :, h * 512 : (h + 1) * 512],
                        start=(u == 0),
                        stop=(u == KU - 1),
                        perf_mode=DR,
                    )

            rln = big.tile([P, RB], F32, name="rln")
            rinvb = big.tile([P, RB], F32, name="rinvb")

            def r_inv():
                # rinvb = 1/(T*||r_j||) = Exp(-0.5 * Ln(T^2 * rsq))
                nc.scalar.activation(
                    out=rln, in_=rsqp, func=AF.Ln, scale=TEMP * TEMP
                )
                nc.scalar.activation(out=rinvb, in_=rln, func=AF.Exp, scale=-0.5)

            # ---- b-shard norms: fused square+reduce, split DVE/ACT.
            ssq_b = big.tile([P, NT], F32, name="ssq_b")
            for t in range(NT):
                btrash = work.tile([P, D], F8, tag="bt", name="btrash")
                with nc.allow_low_precision(reason="fp8 squares"):
                    if t % 2 == 0:
                        nc.vector.scalar_tensor_tensor(
                            out=btrash,
                            in0=bh8[:, t],
                            scalar=1.0,
                            in1=bh8[:, t],
                            op0=ALU.mult,
                            op1=ALU.mult,
                            accum_out=ssq_b[:, t : t + 1],
                        )
                    else:
                        nc.scalar.activation(
                            out=btrash,
                            in_=bh8[:, t],
                            func=AF.Square,
                            accum_out=ssq_b[:, t : t + 1],
                        )
            bln = big.tile([P, NT], F32, name="bln")
            binv = big.tile([P, NT], F32, name="binv")
            nc.scalar.activation(out=bln, in_=ssq_b, func=AF.Ln)
            nc.scalar.activation(out=binv, in_=bln, func=AF.Exp, scale=-0.5)
            binv8 = big.tile([P, NT, 1], F8, name="binv8")
            with nc.allow_low_precision(reason="fp8 matmul weights"):
                nc.scalar.copy(
                    out=binv8, in_=binv.rearrange("p (n o) -> p n o", o=1)
                )

            ssq_s = big.tile([P, SCN], F32, name="ssq_s")
            sln = big.tile([P, SCN], F32, name="sln")
            sinv = big.tile([P, SCN], F32, name="sinv")
            shinv8 = big.tile([P, NT, 1], F8, name="shinv8")

            # ---- numerator partials: out[1, d] = sum_i x[i, d] * inv[i]
            def numerator(x, inv, out_dram, label):
                osb = big.tile([1, D], F32, name=f"osb_{label}")
                for g in range(2):
                    nps = gpool.tile([P, 1024], F32, tag="g", name="nps")
                    for h in range(2):
                        col = g * 1024 + h * 512
                        for t in range(NT):
                            nc.tensor.matmul(
                                nps[0:1, h * 512 : (h + 1) * 512],
                                lhsT=inv[:, t, :],
                                rhs=x[:, t, col : col + 512],
                                start=(t == 0),
                                stop=(t == NT - 1),
                            )
                    if g == 0:
                        nc.scalar.copy(out=osb[:, 0:1024], in_=nps[0:1, :])
                    else:
                        nc.vector.tensor_copy(out=osb[:, 1024:2048], in_=nps[0:1, :])
                nc.sync.dma_start(out=out_dram[:, :], in_=osb)

            # ---- main loop. Chunk sc: 16 DoubleRow sim matmuls into a
            # [128, 1024] psum tile, one DVE multiply by rinvb (bf16 out),
            # one exp with per-partition scale sinv[sc] accumulating the
            # denominator column. The chunk's s-norm square+reduce (tile
            # sc of sm8) is interleaved at the top, alternating DVE/ACT.
            deferred = []
            for sc in range(SCN):
                pr, c = divmod(sc, 2)
                strash = work.tile([P, D], F8, tag="st", name="strash")
                with nc.allow_low_precision(reason="fp8 squares"):
                    if sc % 2 == 0:
                        nc.vector.scalar_tensor_tensor(
                            out=strash,
                            in0=sm8[:, sc],
                            scalar=1.0,
                            in1=sm8[:, sc],
                            op0=ALU.mult,
                            op1=ALU.mult,
                            accum_out=ssq_s[:, sc : sc + 1],
                        )
                    else:
                        nc.scalar.activation(
                            out=strash,
                            in_=sm8[:, sc],
                            func=AF.Square,
                            accum_out=ssq_s[:, sc : sc + 1],
                        )
                nc.scalar.activation(
                    out=sln[:, sc : sc + 1], in_=ssq_s[:, sc : sc + 1], func=AF.Ln
                )
                nc.scalar.activation(
                    out=sinv[:, sc : sc + 1],
                    in_=sln[:, sc : sc + 1],
                    func=AF.Exp,
                    scale=-0.5,
                )

                gts = gpool.tile([P, 1024], F32, tag="g", name=f"g{sc}")
                # the final chunk finishes h0's accumulation first so its
                # mult+exp overlap h1's matmuls, shortening the tail chain
                order = (
                    [(u, h) for h in range(2) for u in range(KU)]
                    if sc == SCN - 1
                    else [(u, h) for u in range(KU) for h in range(2)]
                )
                for u, h in order:
                    w = sT8[:, pr, u, :, c * P : (c + 1) * P]
                    nc.tensor.matmul(
                        gts[:, h * 512 : (h + 1) * 512],
                        lhsT=w,
                        rhs=rT8[:, u, :, h * 512 : (h + 1) * 512],
                        start=(u == 0),
                        stop=(u == KU - 1),
                        perf_mode=DR,
                    )

                def mult_exp(src, col):
                    gs = work.tile([P, 1024], BF16, tag="gs", bufs=3, name="gs")
                    with nc.allow_low_precision(reason="bf16 sims"):
                        nc.vector.tensor_mul(gs, src, rinvb)
                    etrash = work.tile([P, 1024], BF16, tag="esc", name="esc")
                    nc.scalar.activation(
                        out=etrash,
                        in_=gs,
                        func=AF.Exp,
                        scale=sinv[:, col : col + 1],
                        accum_out=dacc[:, col : col + 1],
                    )

                if sc == 1:
                    r_ones(0)
                if sc == 2:
                    r_ones(1)
                    r_inv()

                # The first DEFER chunks evacuate their psum tile as raw
                # bf16 sims so the PE ring never blocks on the rinvb chain;
                # their rinvb multiply + exp run once rinvb exists (flushed
                # two per chunk to avoid a DVE burst).
                if sc < DEFER:
                    raw = work.tile(
                        [P, 1024], BF16, tag="raw", bufs=DEFER, name="raw"
                    )
                    with nc.allow_low_precision(reason="bf16 sims"):
                        nc.vector.tensor_copy(out=raw, in_=gts)
                    deferred.append((raw, sc))
                else:
                    for raw, col in deferred[:2]:
                        mult_exp(raw, col)
                    del deferred[:2]
                    if sc == SCN - 1:
                        for h in range(2):
                            gsh = work.tile(
                                [P, 512], BF16, tag="gsh", name="gsh"
                            )
                            with nc.allow_low_precision(reason="bf16 sims"):
                                nc.vector.tensor_mul(
                                    gsh,
                                    gts[:, h * 512 : (h + 1) * 512],
                                    rinvb[:, h * 512 : (h + 1) * 512],
                                )
                            eth = work.tile(
                                [P, 512], BF16, tag="eth", name="eth"
                            )
                            nc.scalar.activation(
                                out=eth,
                                in_=gsh,
                                func=AF.Exp,
                                scale=sinv[:, sc : sc + 1],
                                accum_out=dacc[:, sc + h : sc + h + 1],
                            )
                    else:
                        mult_exp(gts, sc)

                if sc == 8:
                    with nc.allow_low_precision(reason="fp8 matmul weights"):
                        nc.scalar.copy(
                            out=shinv8,
                            in_=sinv[:, 0:NT].rearrange("p (n o) -> p n o", o=1),
                        )
                if sc == 10:
                    numerator(sh8, shinv8, ssum_d, "s")
                if sc == 12:
                    numerator(bh8, binv8, bsum_d, "b")
                if sc == 13:
                    nc.sync.dma_start(out=dacc_d[:, 0:8], in_=dacc[:, 0:8])

            nc.sync.dma_start(
                out=dacc_d[:, 8 : SCN + 1], in_=dacc[:, 8 : SCN + 1]
            )
    return nc


_NC_CACHE = {}


def _get_nc():
    if "nc" not in _NC_CACHE:
        _NC_CACHE["nc"] = build()
    return _NC_CACHE["nc"]


def _blocked_T(x8):
    """[rows, D] fp8 -> [128, KU, 2, rows] with k = (2u+t)*128 + p."""
    rows = x8.shape[0]
    xT = np.ascontiguousarray(x8.T)  # [D, rows]
    return np.ascontiguousarray(xT.reshape(KU, 2, P, rows).transpose(2, 0, 1, 3))


def _make_in_maps(x_source, x_bc_target, x_raw_target):
    import ml_dtypes

    f8 = ml_dtypes.float8_e4m3
    s8 = np.asarray(x_source, dtype=np.float32).astype(f8)
    r8 = np.asarray(x_raw_target, dtype=np.float32).astype(f8)
    b8 = np.asarray(x_bc_target, dtype=np.float32).astype(f8)

    in_maps = []
    for c in range(8):
        a, b = divmod(c, B_SPLIT)
        sblk = s8[a * SB : (a + 1) * SB]
        # Rotate so the core's numerator shard (local rows b*512..(b+1)*512)
        # lands first; the sim-matrix row permutation leaves the exp-sum
        # unchanged and lets one program serve all cores.
        sblk = np.concatenate(
            [sblk[b * NSH : (b + 1) * NSH], sblk[: b * NSH], sblk[(b + 1) * NSH :]],
            axis=0,
        )
        sT8 = _blocked_T(sblk)  # [128, KU, 2, 2048]
        # pair-chunk blocks: [8, 128, KU, 2, 256]
        sT8b = np.ascontiguousarray(
            sT8.reshape(P, KU, 2, SP2, 2 * P).transpose(3, 0, 1, 2, 4)
        )
        rblk = r8[b * RB : (b + 1) * RB]
        rT8b = np.ascontiguousarray(_blocked_T(rblk).transpose(1, 0, 2, 3))
        in_maps.append(
            {
                "sT8": sT8b,
                "rT8": rT8b,  # [KU, 128, 2, 1024]
                "sm8": np.ascontiguousarray(
                    sblk.reshape(SCN // NT, NT, P, D).transpose(0, 2, 1, 3)
                ),
                "sh8": np.ascontiguousarray(
                    sblk[0:NSH].reshape(NT, P, D).transpose(1, 0, 2)
                ),
                "bh8": np.ascontiguousarray(
                    b8[c * NSH : (c + 1) * NSH].reshape(NT, P, D).transpose(1, 0, 2)
                ),
            }
        )
    return in_maps


def _combine(results):
    denom = 0.0
    s_tot = np.zeros(D, dtype=np.float64)
    b_tot = np.zeros(D, dtype=np.float64)
    for r in results:
        denom += r["dacc"].astype(np.float64).sum()
        s_tot += r["ssum"][0].astype(np.float64)
        b_tot += r["bsum"][0].astype(np.float64)
    loss = np.log(denom) - (s_tot @ b_tot) / (float(N) * float(N)) / TEMP
    return np.array(loss, dtype=np.float32)


def _run(x_source, x_bc_target, x_raw_target, trace=False):
    nc = _get_nc()
    in_maps = _make_in_maps(x_source, x_bc_target, x_raw_target)
    res = run_bass_kernel_spmd(nc, in_maps, core_ids=list(range(8)), trace=trace)
    return _combine(res.results), res


def kernel(x_source, x_bc_target, x_raw_target):
    out, _ = _run(x_source, x_bc_target, x_raw_target)
    return out
